# revision 12
# baseline (speedup 1.0000x reference)
"""Multi-head attention (B=4, T=2048, C=1024, H=16, D=64) on 8 TRN2 cores.

Sharding: core i handles batch b=i//2 and the 8 heads of half hh=i%2.
Each core computes its heads' contribution through the row-sharded output
projection -> partial y [T, C] (fp16); host sums the two partials.

v4: fp16 matmul inputs; x resident in SBUF; host-prearranged contiguous
weight layouts (fast DMAs); pipelined start (qk t-tile 0 only, the rest
of QKV folded into stage 0's interleave with deadlines); fine-grained
emission interleave with j-weighted spreading (late j-tiles are
ACT-heavy and absorb more PE filler) to keep the PE stream dense (HAM
clock gate); multiplicative fp16 causal mask on probabilities;
reciprocal_approx_fast + K=1 fp16 matmul broadcast for the softmax
normalizer; bias folded into the DVE PSUM->SBUF copy; y written fp16,
one DMA per 128-row tile; output projection spread through stage 3.

Per-core layouts (host pre-arranged, fp16):
  xT  [C, T]             = x[b].T
  wq/wk [P, 4, 8, 128]   [sbuf-row, pair, c-chunk, (head-in-pair, d)]
  wv  [P, 8, 512]        [sbuf-row, c-chunk, (head, d)]
  wpt [512, C]           rows = (local head)*64 + d   (= Wp.T row-slice)
  bp  [C]                bias on even cores, zeros on odd
"""

import os
import sys

import numpy as np

for _p in ("/opt/trn_rl_repo", "/root/.axon_site/_ro/trn_rl_repo"):
    if os.path.isdir(_p) and _p not in sys.path:
        sys.path.append(_p)

import concourse.bass as bass
import concourse.bacc as bacc
import concourse.mybir as mybir
import concourse.tile as tile
from concourse.bass_utils import run_bass_kernel_spmd

B, T, C, H, D = 4, 2048, 1024, 16, 64
HL = H // 2          # heads per core
P = 128
NCH = C // P         # 8 c-chunks
NTT = T // 512       # 4 t-tiles of 512
NSB = T // P         # 16 s-blocks of 128
SCALE = 1.0 / 32.0   # 1/sqrt(C)

F32 = mybir.dt.float32
F16 = mybir.dt.float16


def _build(causal: bool) -> bass.Bass:
    nc = bacc.Bacc("TRN2", target_bir_lowering=False, debug=False, num_devices=8)

    xT = nc.dram_tensor("xT", [C, T], F16, kind="ExternalInput").ap()
    wq_d = nc.dram_tensor("wq", [P, 4, NCH, 2 * D], F16, kind="ExternalInput").ap()
    wk_d = nc.dram_tensor("wk", [P, 4, NCH, 2 * D], F16, kind="ExternalInput").ap()
    wv_d = nc.dram_tensor("wv", [P, NCH, HL * D], F16, kind="ExternalInput").ap()
    wpt_d = nc.dram_tensor("wpt", [HL * D, C], F16, kind="ExternalInput").ap()
    bp_d = nc.dram_tensor("bp", [C], F16, kind="ExternalInput").ap()
    y_d = nc.dram_tensor("y", [T, C], F16, kind="ExternalOutput").ap()

    with tile.TileContext(nc) as tc:
        _emit(nc, tc, causal, xT, wq_d, wk_d, wv_d, wpt_d, bp_d, y_d)
    nc.compile()
    return nc


def _emit(nc, tc, causal, xT, wq_d, wk_d, wv_d, wpt_d, bp_d, y_d):
    from contextlib import ExitStack

    ctx = ExitStack()
    with ctx:
        consts = ctx.enter_context(tc.tile_pool(name="consts", bufs=1))
        x_pool = ctx.enter_context(tc.tile_pool(name="xh", bufs=8))
        wq_pool = ctx.enter_context(tc.tile_pool(name="wq", bufs=2))
        wk_pool = ctx.enter_context(tc.tile_pool(name="wk", bufs=2))
        wv_pool = ctx.enter_context(tc.tile_pool(name="wv", bufs=1))
        q_pool = ctx.enter_context(tc.tile_pool(name="qT", bufs=2))
        k_pool = ctx.enter_context(tc.tile_pool(name="kT", bufs=2))
        v_pool = ctx.enter_context(tc.tile_pool(name="v", bufs=1))
        oc_pool = ctx.enter_context(tc.tile_pool(name="outcat", bufs=4))
        p_pool = ctx.enter_context(tc.tile_pool(name="pT", bufs=4))
        rz_pool = ctx.enter_context(tc.tile_pool(name="rzb", bufs=2))
        wpt_pool = ctx.enter_context(tc.tile_pool(name="wpt", bufs=4))
        bpb_pool = ctx.enter_context(tc.tile_pool(name="bpb", bufs=1))
        yt_pool = ctx.enter_context(tc.tile_pool(name="yt", bufs=3))
        psA = ctx.enter_context(tc.tile_pool(name="psA", bufs=2, space="PSUM"))
        psB = ctx.enter_context(tc.tile_pool(name="psB", bufs=4, space="PSUM"))

        # ---- constants ----
        # multiplicative causal mask [128, 2, 128] fp16: 1 where free>=part
        mask01 = None
        if causal:
            mask_f = consts.tile([P, 2, P], F32)
            nc.vector.memset(mask_f, 0.0)
            for _u in range(2):
                nc.gpsimd.affine_select(
                    out=mask_f[:, _u, :], in_=mask_f[:, _u, :],
                    compare_op=mybir.AluOpType.is_ge,
                    fill=-1.0, base=0,
                    pattern=[[1, P]], channel_multiplier=-1,
                )
            # mask_f: 0 on valid, -1 on masked -> mask01 = mask_f + 1
            mask01 = consts.tile([P, 2, P], F16)
            nc.vector.tensor_scalar_add(mask01, mask_f, 1.0)

        ones16 = consts.tile([P, P], F16)
        nc.vector.memset(ones16, 1.0)

        # ---- DMAs: wq0/wk0, x half 0, wv, x half 1, rest ----
        wq_t = [None] * 4
        wk_t = [None] * 4

        def load_wqk(p):
            wq_t[p] = wq_pool.tile([P, NCH, 2 * D], F16, tag="wq", name=f"wq{p}")
            wk_t[p] = wk_pool.tile([P, NCH, 2 * D], F16, tag="wk", name=f"wk{p}")
            nc.sync.dma_start(out=wq_t[p], in_=wq_d[:, p, :, :])
            nc.sync.dma_start(out=wk_t[p], in_=wk_d[:, p, :, :])

        load_wqk(0)
        xh = [x_pool.tile([P, T], F16, tag="xh", name=f"xh{c}") for c in range(NCH)]
        for c in range(NCH):
            nc.gpsimd.dma_start(
                out=xh[c][:, 0:1024], in_=xT[c * P:(c + 1) * P, 0:1024])
        wv_t = wv_pool.tile([P, NCH, HL * D], F16, tag="wv")
        nc.sync.dma_start(out=wv_t, in_=wv_d)
        for c in range(NCH):
            nc.gpsimd.dma_start(
                out=xh[c][:, 1024:2048], in_=xT[c * P:(c + 1) * P, 1024:2048])
        load_wqk(1)
        wpt_t = [wpt_pool.tile([P, C], F16, tag="wpt", name=f"wpt{i}")
                 for i in range(4)]
        for q in range(4):
            nc.sync.dma_start(out=wpt_t[q], in_=wpt_d[q * P:(q + 1) * P, :])
        bpb16 = bpb_pool.tile([P, C], F16)
        nc.sync.dma_start(
            out=bpb16,
            in_=bass.AP(tensor=bp_d.tensor, offset=0, ap=[[0, P], [1, C]]))
        bpb = bpb_pool.tile([P, C], F32)
        nc.vector.tensor_copy(out=bpb, in_=bpb16)

        # persistent SBUF state
        qT = [None] * 4
        kT = [None] * 4
        # v: [s-part, s-block, head, d + ones]
        v_t = v_pool.tile([P, NSB, HL, D + 1], F16, tag="v")
        nc.vector.memset(v_t[:, :, :, D:], 1.0)
        outcat = [oc_pool.tile([P, T], F16, tag="outcat", name=f"outcat{i}")
                  for i in range(4)]

        # ---- emission helpers: each item emits a small group of PE work,
        # drained between attention s-blocks to keep TensorE dense ----
        _stash = {}

        def v_item(vh, sb, chalf):
            def emit():
                if chalf == 0:
                    vps = psB.tile([P, 512], F32, tag="psB", name="vps")
                    _stash[("v", vh, sb)] = vps
                else:
                    vps = _stash.pop(("v", vh, sb))
                csl = slice(vh * 256, vh * 256 + 256)
                for c in range(chalf * 4, chalf * 4 + 4):
                    nc.tensor.matmul(
                        vps[:, 0:256], xh[c][:, sb * P:(sb + 1) * P],
                        wv_t[:, c, csl],
                        start=(c == 0), stop=(c == 7), skip_group_check=True)
                if chalf == 1:
                    nc.vector.tensor_copy(
                        out=v_t[:, sb, vh * 4:vh * 4 + 4, 0:D],
                        in_=vps[:, 0:256].rearrange("p (h d) -> p h d", h=4))
            return emit

        def qk_item(p, tt, which, chalf):
            def emit():
                if chalf == 0:
                    tl = psB.tile([P, 512], F32, tag="psB", name=f"{which}ps")
                    _stash[(p, tt, which)] = tl
                else:
                    tl = _stash.pop((p, tt, which))
                w_t = wq_t[p] if which == "q" else wk_t[p]
                for c in range(chalf * 4, chalf * 4 + 4):
                    nc.tensor.matmul(
                        tl, w_t[:, c, :], xh[c][:, tt * 512:(tt + 1) * 512],
                        start=(c == 0), stop=(c == 7), skip_group_check=True)
                if chalf == 1:
                    dst = qT[p] if which == "q" else kT[p]
                    nc.vector.tensor_copy(
                        out=dst[:, tt * 512:(tt + 1) * 512], in_=tl)
            return emit

        def alloc_qk(p):
            qT[p] = q_pool.tile([P, T], F16, tag="qT", name=f"qT{p}")
            kT[p] = k_pool.tile([P, T], F16, tag="kT", name=f"kT{p}")

        def qk_tt_items(p, tt):
            return [qk_item(p, tt, which, chalf)
                    for which in ("q", "k") for chalf in range(2)]

        def proj_item(m):
            def emit():
                yt = yt_pool.tile([P, C], F16, tag="yt", name="yt")
                for n in range(2):
                    yps = psB.tile([P, 512], F32, tag="psB", name="yps")
                    for q in range(4):
                        nc.tensor.matmul(
                            yps,
                            outcat[q][:, m * P:(m + 1) * P],
                            wpt_t[q][:, n * 512:(n + 1) * 512],
                            start=(q == 0), stop=(q == 3),
                            skip_group_check=True)
                    nc.vector.tensor_add(
                        yt[:, n * 512:(n + 1) * 512], yps,
                        bpb[:, n * 512:(n + 1) * 512])
                nc.gpsimd.dma_start(out=y_d[m * P:(m + 1) * P, :], in_=yt)
            return emit

        # ---- preamble: qk(0) t-tile 0, v s-blocks 0-3 ----
        alloc_qk(0)
        for it in qk_tt_items(0, 0):
            it()
        for sb in range(4):
            for chalf in range(2):
                v_item(0, sb, chalf)()

        # ---- pair stages: attention(p) interleaved with deadline-
        # scheduled fill work (rest of QKV, next pair's qk, projection) ----
        deferred = []
        for p in range(4):
            # block index layout of this stage
            nsb_js = [(4 * (j + 1) if causal else NSB) for j in range(NTT)]
            jstart = [sum(nsb_js[:j]) for j in range(NTT)]
            nblocks = sum(nsb_js)

            # fill items: (deadline_block, emit_fn); deadline = last block
            # index by which the item must have been emitted
            ilv = []
            if p == 0:
                for j in range(1, NTT):
                    for it in qk_tt_items(0, j):
                        ilv.append((jstart[j] - 1, it))
                for sb in range(4, NSB):
                    dl = jstart[sb // 4] + sb - 1
                    for chalf in range(2):
                        ilv.append((dl, v_item(0, sb, chalf)))
            if p == 1:
                for sb in range(NSB):
                    for chalf in range(2):
                        ilv.append((nblocks, v_item(1, sb, chalf)))
            if p < 3:
                alloc_qk(p + 1)
                for j in range(NTT):
                    for it in qk_tt_items(p + 1, j):
                        ilv.append((nblocks, it))
                if p < 2:
                    ilv.append((nblocks, lambda pp=p + 2: load_wqk(pp)))
            ilv.sort(key=lambda e: e[0])

            # weighted spreading: block in j-tile j has weight j+1 (late
            # tiles are ACT-heavy and can absorb more PE filler)
            wts = []
            for j in range(NTT):
                wts += [j + 1] * nsb_js[j]
            wtot = sum(wts)
            wcum = 0.0
            drained = [0]

            def drain_for_block(b, ilv=ilv, drained=drained, wts=wts,
                                wtot=wtot):
                nonlocal wcum
                wcum += wts[b] if b < len(wts) else 0
                n0 = len(ilv) + drained[0]
                while ilv and (ilv[0][0] <= b + 1
                               or drained[0] + 1 <= n0 * wcum / wtot):
                    ilv.pop(0)[1]()
                    drained[0] += 1

            bcnt = 0
            # Z layout: row 32*u, col j (rows 0/32 are valid PE base
            # partitions for the K=1 broadcast matmul)
            zpair = rz_pool.tile([P, NTT, 512], F32, tag="rzb", name="zpair")
            zscr = rz_pool.tile([P, NTT, 512], F32, tag="zscr", name="zscr")
            zr16 = rz_pool.tile([P, NTT, 512], F16, tag="zr16", name="zr16")

            def normalize(j, p=p, zr16=zr16):
                for u in range(2):
                    k0 = 32 * u
                    bps = psB.tile([P, 512], F32, tag="psB", name="bps")
                    nc.tensor.matmul(
                        bps, ones16[k0:k0 + 1, :],
                        zr16[k0:k0 + 1, j, :],
                        start=True, stop=True)
                    osl = outcat[p][u * D:(u + 1) * D,
                                    j * 512:(j + 1) * 512]
                    nc.vector.tensor_mul(osl, osl, bps[0:D, :])

            for j in range(NTT):
                nsb_j = nsb_js[j]
                outp = [psB.tile([D + 1, 512], F32, tag="psB",
                                 name=f"outp{u}") for u in range(2)]

                pend = {}

                def emit_pv(i, lo, last, outp=outp, p=p, pend=pend):
                    pts = pend.pop(i)
                    for u in range(2):
                        nc.tensor.matmul(
                            outp[u][:, lo:512],
                            v_t[:, i, p * 2 + u, :],
                            pts[:, u, lo:512],
                            start=(i == 0), stop=last,
                            skip_group_check=True)

                prev = None
                for i in range(nsb_j):
                    drain_for_block(bcnt)
                    bcnt += 1
                    r = i - 4 * j if causal else -1
                    lo = max(r, 0) * P
                    scs = psA.tile([P, 2, 512], F32, tag="psA", name="scs")
                    pts = p_pool.tile([P, 2, 512], F16, tag="pT", name="pts")
                    pend[i] = pts
                    for u in range(2):
                        dsl = slice(u * D, (u + 1) * D)
                        nc.tensor.matmul(
                            scs[:, u, lo:512],
                            kT[p][dsl, i * P:(i + 1) * P],
                            qT[p][dsl, j * 512 + lo:(j + 1) * 512],
                            start=True, stop=True)
                    nc.scalar.activation(
                        out=pts[:, :, lo:512],
                        in_=scs[:, :, lo:512],
                        func=mybir.ActivationFunctionType.Exp,
                        scale=SCALE)
                    if causal and r >= 0:
                        nc.vector.tensor_mul(
                            pts[:, :, lo:lo + P],
                            pts[:, :, lo:lo + P],
                            mask01)
                    if deferred:
                        if i == 2:
                            deferred.pop(0)()
                            if p == 3 and deferred:
                                deferred.pop(0)()
                        elif p == 3 and i == 5 and deferred:
                            deferred.pop(0)()
                    if prev is not None:
                        emit_pv(*prev)
                    prev = (i, lo, i == nsb_j - 1)
                emit_pv(*prev)

                # raw head output + Z row out of PSUM (releases outp ring)
                for u in range(2):
                    nc.vector.tensor_copy(
                        out=outcat[p][u * D:(u + 1) * D,
                                      j * 512:(j + 1) * 512],
                        in_=outp[u][0:D, :])
                    nc.vector.tensor_copy(
                        out=zpair[32 * u:32 * u + 1, j, :],
                        in_=outp[u][D:D + 1, :])

                # per-j normalizer chain (keeps the DVE work smooth and,
                # for pair 3, lets the projection start tile-by-tile)
                with nc.allow_low_precision(reason="softmax normalizer"):
                    nc.vector.reciprocal_approx_fast(
                        out=zscr[0:64, j, :], in_=zpair[0:64, j, :])
                    nc.vector.tensor_copy(
                        out=zr16[0:64, j, :], in_=zscr[0:64, j, :])

                if p == 3:
                    def norm3(j=j, ilv=ilv, nf=normalize):
                        nf(j)
                        for m in range(4 * j, 4 * j + 4):
                            ilv.append((10 ** 9, proj_item(m)))
                    deferred.append(norm3)
                else:
                    deferred.append(lambda j=j, nf=normalize: nf(j))

            if p < 3:
                while ilv:
                    ilv.pop(0)[1]()
            else:
                for fn in deferred:
                    fn()
                deferred = []
                while ilv:
                    ilv.pop(0)[1]()


_NC_CACHE = {}
LAST_RESULTS = None


def kernel(x, Wq, Wk, Wv, Wp, bp, is_masked, **_unused):
    global LAST_RESULTS
    x = np.asarray(x, np.float32)
    Wq = np.asarray(Wq, np.float32)
    Wk = np.asarray(Wk, np.float32)
    Wv = np.asarray(Wv, np.float32)
    Wp = np.asarray(Wp, np.float32)
    bp = np.asarray(bp, np.float32)
    causal = bool(np.asarray(is_masked).item())

    if causal not in _NC_CACHE:
        _NC_CACHE[causal] = _build(causal)
    nc = _NC_CACHE[causal]

    # host-side layout prep (fp16, contiguous per-partition DMAs)
    wq_r = np.ascontiguousarray(Wq.transpose(1, 0, 2).reshape(C, H * D)).astype(np.float16)
    wk_r = np.ascontiguousarray(Wk.transpose(1, 0, 2).reshape(C, H * D)).astype(np.float16)
    wv_r = np.ascontiguousarray(Wv.transpose(1, 0, 2).reshape(C, H * D)).astype(np.float16)
    wpt = np.ascontiguousarray(Wp.T).astype(np.float16)
    bp16 = bp.astype(np.float16)
    zeros = np.zeros_like(bp16)

    xTs = [np.ascontiguousarray(x[b].T).astype(np.float16) for b in range(B)]
    in_maps = []
    for core in range(8):
        b, hh = core // 2, core % 2
        csl = slice(hh * HL * D, (hh + 1) * HL * D)
        # [C, 512] -> [p, pair, chunk, (head-in-pair, d)]
        wq_c = np.ascontiguousarray(
            wq_r[:, csl].reshape(NCH, P, 4, 2 * D).transpose(1, 2, 0, 3))
        wk_c = np.ascontiguousarray(
            wk_r[:, csl].reshape(NCH, P, 4, 2 * D).transpose(1, 2, 0, 3))
        # [C, 512] -> [p, chunk, (head, d)]
        wv_c = np.ascontiguousarray(
            wv_r[:, csl].reshape(NCH, P, HL * D).transpose(1, 0, 2))
        in_maps.append({
            "xT": xTs[b],
            "wq": wq_c,
            "wk": wk_c,
            "wv": wv_c,
            "wpt": np.ascontiguousarray(wpt[csl, :]),
            "bp": bp16 if hh == 0 else zeros,
        })

    trace = bool(int(os.environ.get("KERNEL_TRACE", "0")))
    res = run_bass_kernel_spmd(
        nc, in_maps, core_ids=list(range(8)), trace=trace)
    LAST_RESULTS = res

    y = np.empty((B, T, C), np.float32)
    for b in range(B):
        y[b] = (res.results[2 * b]["y"].astype(np.float32)
                + res.results[2 * b + 1]["y"].astype(np.float32))
    return y


# revision 13
# speedup vs baseline: 1.0432x; 1.0432x over previous
"""Multi-head attention (B=4, T=2048, C=1024, H=16, D=64) on 8 TRN2 cores.

Sharding: core i handles batch b=i//2 and the 8 heads of half hh=i%2.
Each core computes its heads' contribution through the row-sharded output
projection -> partial y [T, C] (fp16); host sums the two partials.

v4: fp16 matmul inputs; x resident in SBUF; host-prearranged contiguous
weight layouts (fast DMAs); pipelined start (qk t-tile 0 only, the rest
of QKV folded into stage 0's interleave with deadlines); fine-grained
emission interleave with j-weighted spreading (late j-tiles are
ACT-heavy and absorb more PE filler) to keep the PE stream dense (HAM
clock gate); multiplicative fp16 causal mask on probabilities;
reciprocal_approx_fast + K=1 fp16 matmul broadcast for the softmax
normalizer; bias folded into the DVE PSUM->SBUF copy; y written fp16,
one DMA per 128-row tile; output projection spread through stage 3.

Per-core layouts (host pre-arranged, fp16):
  xT  [C, T]             = x[b].T
  wq/wk [P, 4, 8, 128]   [sbuf-row, pair, c-chunk, (head-in-pair, d)]
  wv  [P, 8, 512]        [sbuf-row, c-chunk, (head, d)]
  wpt [512, C]           rows = (local head)*64 + d   (= Wp.T row-slice)
  bp  [C]                bias on even cores, zeros on odd
"""

import os
import sys

import numpy as np

for _p in ("/opt/trn_rl_repo", "/root/.axon_site/_ro/trn_rl_repo"):
    if os.path.isdir(_p) and _p not in sys.path:
        sys.path.append(_p)

import concourse.bass as bass
import concourse.bacc as bacc
import concourse.mybir as mybir
import concourse.tile as tile
from concourse.bass_utils import run_bass_kernel_spmd

B, T, C, H, D = 4, 2048, 1024, 16, 64
HL = H // 2          # heads per core
P = 128
NCH = C // P         # 8 c-chunks
NTT = T // 512       # 4 t-tiles of 512
NSB = T // P         # 16 s-blocks of 128
SCALE = 1.0 / 32.0   # 1/sqrt(C)

F32 = mybir.dt.float32
F16 = mybir.dt.float16


def _build(causal: bool) -> bass.Bass:
    nc = bacc.Bacc("TRN2", target_bir_lowering=False, debug=False, num_devices=8)

    xT = nc.dram_tensor("xT", [C, T], F16, kind="ExternalInput").ap()
    wq_d = nc.dram_tensor("wq", [P, 4, NCH, 2 * D], F16, kind="ExternalInput").ap()
    wk_d = nc.dram_tensor("wk", [P, 4, NCH, 2 * D], F16, kind="ExternalInput").ap()
    wv_d = nc.dram_tensor("wv", [P, NCH, HL * D], F16, kind="ExternalInput").ap()
    wpt_d = nc.dram_tensor("wpt", [HL * D, C], F16, kind="ExternalInput").ap()
    bp_d = nc.dram_tensor("bp", [C], F16, kind="ExternalInput").ap()
    y_d = nc.dram_tensor("y", [T, C], F16, kind="ExternalOutput").ap()

    with tile.TileContext(nc) as tc:
        _emit(nc, tc, causal, xT, wq_d, wk_d, wv_d, wpt_d, bp_d, y_d)
    nc.compile()
    return nc


def _emit(nc, tc, causal, xT, wq_d, wk_d, wv_d, wpt_d, bp_d, y_d):
    from contextlib import ExitStack

    ctx = ExitStack()
    with ctx:
        consts = ctx.enter_context(tc.tile_pool(name="consts", bufs=1))
        x_pool = ctx.enter_context(tc.tile_pool(name="xh", bufs=8))
        wq_pool = ctx.enter_context(tc.tile_pool(name="wq", bufs=2))
        wk_pool = ctx.enter_context(tc.tile_pool(name="wk", bufs=2))
        wv_pool = ctx.enter_context(tc.tile_pool(name="wv", bufs=1))
        q_pool = ctx.enter_context(tc.tile_pool(name="qT", bufs=2))
        k_pool = ctx.enter_context(tc.tile_pool(name="kT", bufs=2))
        v_pool = ctx.enter_context(tc.tile_pool(name="v", bufs=1))
        oc_pool = ctx.enter_context(tc.tile_pool(name="outcat", bufs=4))
        p_pool = ctx.enter_context(tc.tile_pool(name="pT", bufs=4))
        rz_pool = ctx.enter_context(tc.tile_pool(name="rzb", bufs=2))
        wpt_pool = ctx.enter_context(tc.tile_pool(name="wpt", bufs=4))
        bpb_pool = ctx.enter_context(tc.tile_pool(name="bpb", bufs=1))
        yt_pool = ctx.enter_context(tc.tile_pool(name="yt", bufs=3))
        psA = ctx.enter_context(tc.tile_pool(name="psA", bufs=2, space="PSUM"))
        psB = ctx.enter_context(tc.tile_pool(name="psB", bufs=4, space="PSUM"))

        # ---- constants ----
        # multiplicative causal mask [128, 2, 128] fp16: 1 where free>=part
        mask01 = None
        if causal:
            mask_f = consts.tile([P, 2, P], F32)
            nc.vector.memset(mask_f, 0.0)
            for _u in range(2):
                nc.gpsimd.affine_select(
                    out=mask_f[:, _u, :], in_=mask_f[:, _u, :],
                    compare_op=mybir.AluOpType.is_ge,
                    fill=-1.0, base=0,
                    pattern=[[1, P]], channel_multiplier=-1,
                )
            # mask_f: 0 on valid, -1 on masked -> mask01 = mask_f + 1
            mask01 = consts.tile([P, 2, P], F16)
            nc.vector.tensor_scalar_add(mask01, mask_f, 1.0)

        ones16 = consts.tile([P, P], F16)
        nc.vector.memset(ones16, 1.0)

        # ---- DMAs: wq0/wk0, x half 0, wv, x half 1, rest ----
        wq_t = [None] * 4
        wk_t = [None] * 4

        def load_wqk(p):
            wq_t[p] = wq_pool.tile([P, NCH, 2 * D], F16, tag="wq", name=f"wq{p}")
            wk_t[p] = wk_pool.tile([P, NCH, 2 * D], F16, tag="wk", name=f"wk{p}")
            nc.sync.dma_start(out=wq_t[p], in_=wq_d[:, p, :, :])
            nc.sync.dma_start(out=wk_t[p], in_=wk_d[:, p, :, :])

        load_wqk(0)
        xh = [x_pool.tile([P, T], F16, tag="xh", name=f"xh{c}") for c in range(NCH)]
        for c in range(NCH):
            nc.sync.dma_start(
                out=xh[c][:, 0:1024], in_=xT[c * P:(c + 1) * P, 0:1024])
        wv_t = wv_pool.tile([P, NCH, HL * D], F16, tag="wv")
        nc.sync.dma_start(out=wv_t, in_=wv_d)
        for c in range(NCH):
            nc.sync.dma_start(
                out=xh[c][:, 1024:2048], in_=xT[c * P:(c + 1) * P, 1024:2048])
        load_wqk(1)
        wpt_t = [wpt_pool.tile([P, C], F16, tag="wpt", name=f"wpt{i}")
                 for i in range(4)]
        for q in range(4):
            nc.sync.dma_start(out=wpt_t[q], in_=wpt_d[q * P:(q + 1) * P, :])
        bpb16 = bpb_pool.tile([P, C], F16)
        nc.sync.dma_start(
            out=bpb16,
            in_=bass.AP(tensor=bp_d.tensor, offset=0, ap=[[0, P], [1, C]]))
        bpb = bpb_pool.tile([P, C], F32)
        nc.vector.tensor_copy(out=bpb, in_=bpb16)

        # persistent SBUF state
        qT = [None] * 4
        kT = [None] * 4
        # v: [s-part, s-block, head, d + ones]
        v_t = v_pool.tile([P, NSB, HL, D + 1], F16, tag="v")
        nc.vector.memset(v_t[:, :, :, D:], 1.0)
        outcat = [oc_pool.tile([P, T], F16, tag="outcat", name=f"outcat{i}")
                  for i in range(4)]

        # ---- emission helpers: each item emits a small group of PE work,
        # drained between attention s-blocks to keep TensorE dense ----
        _stash = {}

        def v_item(sb, chalf):
            def emit():
                if chalf == 0:
                    vps = psB.tile([P, HL * D], F32, tag="psB", name="vps")
                    _stash[("v", sb)] = vps
                else:
                    vps = _stash.pop(("v", sb))
                for c in range(chalf * 4, chalf * 4 + 4):
                    nc.tensor.matmul(
                        vps, xh[c][:, sb * P:(sb + 1) * P], wv_t[:, c, :],
                        start=(c == 0), stop=(c == 7), skip_group_check=True)
                if chalf == 1:
                    nc.vector.tensor_copy(
                        out=v_t[:, sb, :, 0:D],
                        in_=vps.rearrange("p (h d) -> p h d", h=HL))
            return emit

        def qk_item(p, tt, which, chalf):
            def emit():
                if chalf == 0:
                    tl = psB.tile([P, 512], F32, tag="psB", name=f"{which}ps")
                    _stash[(p, tt, which)] = tl
                else:
                    tl = _stash.pop((p, tt, which))
                w_t = wq_t[p] if which == "q" else wk_t[p]
                for c in range(chalf * 4, chalf * 4 + 4):
                    nc.tensor.matmul(
                        tl, w_t[:, c, :], xh[c][:, tt * 512:(tt + 1) * 512],
                        start=(c == 0), stop=(c == 7), skip_group_check=True)
                if chalf == 1:
                    dst = qT[p] if which == "q" else kT[p]
                    nc.vector.tensor_copy(
                        out=dst[:, tt * 512:(tt + 1) * 512], in_=tl)
            return emit

        def alloc_qk(p):
            qT[p] = q_pool.tile([P, T], F16, tag="qT", name=f"qT{p}")
            kT[p] = k_pool.tile([P, T], F16, tag="kT", name=f"kT{p}")

        def qk_tt_items(p, tt):
            return [qk_item(p, tt, which, chalf)
                    for which in ("q", "k") for chalf in range(2)]

        def proj_item(m):
            def emit():
                yt = yt_pool.tile([P, C], F16, tag="yt", name="yt")
                for n in range(2):
                    yps = psB.tile([P, 512], F32, tag="psB", name="yps")
                    for q in range(4):
                        nc.tensor.matmul(
                            yps,
                            outcat[q][:, m * P:(m + 1) * P],
                            wpt_t[q][:, n * 512:(n + 1) * 512],
                            start=(q == 0), stop=(q == 3),
                            skip_group_check=True)
                    nc.vector.tensor_add(
                        yt[:, n * 512:(n + 1) * 512], yps,
                        bpb[:, n * 512:(n + 1) * 512])
                nc.sync.dma_start(out=y_d[m * P:(m + 1) * P, :], in_=yt)
            return emit

        # ---- preamble: qk(0) t-tile 0, v s-blocks 0-3 ----
        alloc_qk(0)
        for it in qk_tt_items(0, 0):
            it()
        for sb in range(4):
            for chalf in range(2):
                v_item(sb, chalf)()

        # ---- pair stages: attention(p) interleaved with deadline-
        # scheduled fill work (rest of QKV, next pair's qk, projection) ----
        deferred = []
        for p in range(4):
            # block index layout of this stage
            nsb_js = [(4 * (j + 1) if causal else NSB) for j in range(NTT)]
            jstart = [sum(nsb_js[:j]) for j in range(NTT)]
            nblocks = sum(nsb_js)

            # fill items: (deadline_block, emit_fn); deadline = last block
            # index by which the item must have been emitted
            ilv = []
            if p == 0:
                for j in range(1, NTT):
                    for it in qk_tt_items(0, j):
                        ilv.append((jstart[j] - 1, it))
                for sb in range(4, NSB):
                    dl = jstart[sb // 4] + sb - 1
                    for chalf in range(2):
                        ilv.append((dl, v_item(sb, chalf)))
            if p < 3:
                alloc_qk(p + 1)
                for j in range(NTT):
                    for it in qk_tt_items(p + 1, j):
                        ilv.append((nblocks, it))
                if p < 2:
                    ilv.append((nblocks, lambda pp=p + 2: load_wqk(pp)))
            ilv.sort(key=lambda e: e[0])

            # weighted spreading: block in j-tile j has weight j+1 (late
            # tiles are ACT-heavy and can absorb more PE filler)
            wts = []
            for j in range(NTT):
                wts += [j + 1] * nsb_js[j]
            wtot = sum(wts)
            wcum = 0.0
            drained = [0]

            def drain_for_block(b, ilv=ilv, drained=drained, wts=wts,
                                wtot=wtot):
                nonlocal wcum
                wcum += wts[b] if b < len(wts) else 0
                n0 = len(ilv) + drained[0]
                while ilv and (ilv[0][0] <= b + 1
                               or drained[0] + 1 <= n0 * wcum / wtot):
                    ilv.pop(0)[1]()
                    drained[0] += 1

            bcnt = 0
            # Z layout: row 32*u, col j (rows 0/32 are valid PE base
            # partitions for the K=1 broadcast matmul)
            zpair = rz_pool.tile([P, NTT, 512], F32, tag="rzb", name="zpair")
            zscr = rz_pool.tile([P, NTT, 512], F32, tag="zscr", name="zscr")
            zr16 = rz_pool.tile([P, NTT, 512], F16, tag="zr16", name="zr16")

            def normalize(j, p=p, zr16=zr16):
                for u in range(2):
                    k0 = 32 * u
                    bps = psB.tile([P, 512], F32, tag="psB", name="bps")
                    nc.tensor.matmul(
                        bps, ones16[k0:k0 + 1, :],
                        zr16[k0:k0 + 1, j, :],
                        start=True, stop=True)
                    osl = outcat[p][u * D:(u + 1) * D,
                                    j * 512:(j + 1) * 512]
                    nc.vector.tensor_mul(osl, osl, bps[0:D, :])

            for j in range(NTT):
                nsb_j = nsb_js[j]
                outp = [psB.tile([D + 1, 512], F32, tag="psB",
                                 name=f"outp{u}") for u in range(2)]

                pend = {}

                def emit_pv(i, lo, last, outp=outp, p=p, pend=pend):
                    pts = pend.pop(i)
                    for u in range(2):
                        nc.tensor.matmul(
                            outp[u][:, lo:512],
                            v_t[:, i, p * 2 + u, :],
                            pts[:, u, lo:512],
                            start=(i == 0), stop=last,
                            skip_group_check=True)

                prev = None
                for i in range(nsb_j):
                    drain_for_block(bcnt)
                    bcnt += 1
                    r = i - 4 * j if causal else -1
                    lo = max(r, 0) * P
                    scs = psA.tile([P, 2, 512], F32, tag="psA", name="scs")
                    pts = p_pool.tile([P, 2, 512], F16, tag="pT", name="pts")
                    pend[i] = pts
                    for u in range(2):
                        dsl = slice(u * D, (u + 1) * D)
                        nc.tensor.matmul(
                            scs[:, u, lo:512],
                            kT[p][dsl, i * P:(i + 1) * P],
                            qT[p][dsl, j * 512 + lo:(j + 1) * 512],
                            start=True, stop=True)
                    nc.scalar.activation(
                        out=pts[:, :, lo:512],
                        in_=scs[:, :, lo:512],
                        func=mybir.ActivationFunctionType.Exp,
                        scale=SCALE)
                    if causal and r >= 0:
                        nc.vector.tensor_mul(
                            pts[:, :, lo:lo + P],
                            pts[:, :, lo:lo + P],
                            mask01)
                    if deferred:
                        if i == 2:
                            deferred.pop(0)()
                            if p == 3 and deferred:
                                deferred.pop(0)()
                        elif p == 3 and i == 5 and deferred:
                            deferred.pop(0)()
                    if prev is not None:
                        emit_pv(*prev)
                    prev = (i, lo, i == nsb_j - 1)
                emit_pv(*prev)

                # raw head output + Z row out of PSUM (releases outp ring)
                for u in range(2):
                    nc.vector.tensor_copy(
                        out=outcat[p][u * D:(u + 1) * D,
                                      j * 512:(j + 1) * 512],
                        in_=outp[u][0:D, :])
                    nc.vector.tensor_copy(
                        out=zpair[32 * u:32 * u + 1, j, :],
                        in_=outp[u][D:D + 1, :])

                # per-j normalizer chain (keeps DVE smooth; for pair 3
                # it lets the projection start tile-by-tile in-stage)
                with nc.allow_low_precision(reason="softmax normalizer"):
                    nc.vector.reciprocal_approx_fast(
                        out=zscr[0:64, j, :], in_=zpair[0:64, j, :])
                    nc.vector.tensor_copy(
                        out=zr16[0:64, j, :], in_=zscr[0:64, j, :])

                if p == 3:
                    def norm3(j=j, ilv=ilv, nf=normalize):
                        nf(j)
                        for m in range(4 * j, 4 * j + 4):
                            ilv.append((10 ** 9, proj_item(m)))
                    deferred.append(norm3)
                else:
                    deferred.append(lambda j=j, nf=normalize: nf(j))

            if p < 3:
                while ilv:
                    ilv.pop(0)[1]()
            else:
                for fn in deferred:
                    fn()
                deferred = []
                while ilv:
                    ilv.pop(0)[1]()


_NC_CACHE = {}
LAST_RESULTS = None


def kernel(x, Wq, Wk, Wv, Wp, bp, is_masked, **_unused):
    global LAST_RESULTS
    x = np.asarray(x, np.float32)
    Wq = np.asarray(Wq, np.float32)
    Wk = np.asarray(Wk, np.float32)
    Wv = np.asarray(Wv, np.float32)
    Wp = np.asarray(Wp, np.float32)
    bp = np.asarray(bp, np.float32)
    causal = bool(np.asarray(is_masked).item())

    if causal not in _NC_CACHE:
        _NC_CACHE[causal] = _build(causal)
    nc = _NC_CACHE[causal]

    # host-side layout prep (fp16, contiguous per-partition DMAs)
    wq_r = np.ascontiguousarray(Wq.transpose(1, 0, 2).reshape(C, H * D)).astype(np.float16)
    wk_r = np.ascontiguousarray(Wk.transpose(1, 0, 2).reshape(C, H * D)).astype(np.float16)
    wv_r = np.ascontiguousarray(Wv.transpose(1, 0, 2).reshape(C, H * D)).astype(np.float16)
    wpt = np.ascontiguousarray(Wp.T).astype(np.float16)
    bp16 = bp.astype(np.float16)
    zeros = np.zeros_like(bp16)

    xTs = [np.ascontiguousarray(x[b].T).astype(np.float16) for b in range(B)]
    in_maps = []
    for core in range(8):
        b, hh = core // 2, core % 2
        csl = slice(hh * HL * D, (hh + 1) * HL * D)
        # [C, 512] -> [p, pair, chunk, (head-in-pair, d)]
        wq_c = np.ascontiguousarray(
            wq_r[:, csl].reshape(NCH, P, 4, 2 * D).transpose(1, 2, 0, 3))
        wk_c = np.ascontiguousarray(
            wk_r[:, csl].reshape(NCH, P, 4, 2 * D).transpose(1, 2, 0, 3))
        # [C, 512] -> [p, chunk, (head, d)]
        wv_c = np.ascontiguousarray(
            wv_r[:, csl].reshape(NCH, P, HL * D).transpose(1, 0, 2))
        in_maps.append({
            "xT": xTs[b],
            "wq": wq_c,
            "wk": wk_c,
            "wv": wv_c,
            "wpt": np.ascontiguousarray(wpt[csl, :]),
            "bp": bp16 if hh == 0 else zeros,
        })

    trace = bool(int(os.environ.get("KERNEL_TRACE", "0")))
    res = run_bass_kernel_spmd(
        nc, in_maps, core_ids=list(range(8)), trace=trace)
    LAST_RESULTS = res

    y = np.empty((B, T, C), np.float32)
    for b in range(B):
        y[b] = (res.results[2 * b]["y"].astype(np.float32)
                + res.results[2 * b + 1]["y"].astype(np.float32))
    return y
